# revision 32
# baseline (speedup 1.0000x reference)
"""Trainium2 Bass kernel for the 81-step LSTM decoder + masked softmax.

Math (per batch row b):
    z_t = x_t @ W_x + h_{t-1} @ W_h + b          (gates i, f, g, o; 100 each)
    i,f,o = sigmoid;  g = identity
    c_t = f*c_{t-1} + i*g;  h_t = o*c_t
    out_t = softmax(where(mask_t, h_t, -inf))

Strategy: data-parallel over batch (4096 -> 8 cores x 512). Each core runs
an identical Bass program on its shard; no collectives.

Device layout is feature-major ("transposed"): the recurrent state h^T is
kept as [101, 512] bf16 (hidden-on-partitions, batch-on-free, +1 ones row so
the bias rides in an augmented weight row).  x is fed to the device already
transposed on the host as xT [81, 512e, 512b] so the contraction dim (e)
lands on partitions with zero on-device data transposes.  All z matmuls run
in bf16 (2 cols/cycle on the PE).  The softmax mask and the softmax row-sum
are folded into the per-step PE transpose: instead of an identity, the
transpose right-multiplies by [diag(mask_t) | mask_t], so the PE emits the
masked exp(h) batch-major AND its row sums (column 100) in one pass --
no mask bias on the exp and no vector-engine reduction.  Elementwise work
is split between the Vector and GpSimd engines.
"""

import sys

if "/opt/trn_rl_repo" not in sys.path:
    sys.path.insert(0, "/opt/trn_rl_repo")

import numpy as np

P = 81       # places / timesteps
H = 100      # LSTM units
E = 512      # encoder feature width
B = 4096     # total batch
NCORES = 8
BS = B // NCORES          # 512 batch rows per core
NB = BS // 128            # 4 batch tiles of 128
NE = E // 128             # 4 feature chunks of 128
K = 9                     # softmax/exp batching window (81 % 9 == 0)
MT = 101                  # masked-transpose matrix cols (100 + sum col)
# fp8-e4m3 DoubleRow for the x@W_x matmuls (2 k-tiles/instr, 2x col rate).
# Plain fp8 on every gate fails the 2e-2 gate (2.25e-2); with i,f,o in plain
# fp8 (sigmoid-damped) and the identity gate g residual-corrected
# (x8@W8 + r8@W8 + x8@S8), the simulated/measured rel err is ~1.47e-2.
USE_FP8 = True

_PROGRAM = None


def _build_program():
    import concourse.bacc as bacc
    import concourse.bass as bass
    import concourse.mybir as mybir
    from concourse.tile import TileContext
    from concourse.tile_rust import add_dep_helper
    from contextlib import ExitStack

    f32 = mybir.dt.float32
    bf16 = mybir.dt.bfloat16
    xdt = mybir.dt.float8e4 if USE_FP8 else bf16
    TANH = mybir.ActivationFunctionType.Tanh
    EXP = mybir.ActivationFunctionType.Exp
    ADD = mybir.AluOpType.add
    MULT = mybir.AluOpType.mult

    DR = mybir.MatmulPerfMode.DoubleRow if USE_FP8 else None
    XE = 2 * E if USE_FP8 else E          # x8 rows then r8 rows when fp8
    WCOL = 512 if USE_FP8 else 400        # W8 [0:400], S8_g [400:500], pad

    nc = bacc.Bacc(None, target_bir_lowering=False)

    xT_d = nc.dram_tensor("xT", [P, XE, BS], xdt, kind="ExternalInput")
    wxb_d = nc.dram_tensor("wxb", [128, NE * WCOL], xdt, kind="ExternalInput")
    whb_d = nc.dram_tensor("whb", [H + 1, 400], bf16, kind="ExternalInput")
    mt_d = nc.dram_tensor("mt", [H, P * MT], bf16, kind="ExternalInput")
    h0T_d = nc.dram_tensor("h0T", [H + 1, BS], bf16, kind="ExternalInput")
    out_d = nc.dram_tensor("out", [BS, P, H], f32, kind="ExternalOutput")

    with ExitStack() as ctx:
        tc = ctx.enter_context(TileContext(nc))
        consts = ctx.enter_context(tc.tile_pool(name="consts", bufs=1))
        xpool = ctx.enter_context(tc.tile_pool(name="xpool", bufs=12))
        gpool = ctx.enter_context(tc.tile_pool(name="gpool", bufs=2))
        opool = ctx.enter_context(tc.tile_pool(name="opool", bufs=8))
        zpool = ctx.enter_context(tc.tile_pool(name="zpool", bufs=6, space="PSUM"))
        epool = ctx.enter_context(tc.tile_pool(name="epool", bufs=2, space="PSUM"))

        wxb = consts.tile([128, NE, WCOL], xdt)
        nc.sync.dma_start(out=wxb, in_=wxb_d.rearrange("p (c w) -> p c w", w=WCOL))
        whb = consts.tile([H + 1, 400], bf16)
        nc.sync.dma_start(out=whb, in_=whb_d[:, :])
        mt = consts.tile([H, P, MT], bf16)
        nc.sync.dma_start(out=mt, in_=mt_d.rearrange("h (p m) -> h p m", m=MT))
        # ring of recurrent-state snapshots; row H holds the constant 1.0 that
        # multiplies the bias row of whb
        hist = [consts.tile([H + 1, BS], bf16, name=f"hist{j}") for j in range(K)]
        for j in range(K - 1):
            nc.sync.dma_start(out=hist[j][H : H + 1, :], in_=h0T_d[H : H + 1, :])
        nc.sync.dma_start(out=hist[K - 1], in_=h0T_d[:, :])
        # persistent cell state, stored scaled: sT = 2*c (bf16)
        sT = consts.tile([H, BS], bf16)
        nc.vector.memset(sT, 0.0)

        # W column order: i [0:100], f [100:200], g [200:300], o [300:400]
        GSLICE = (1, 0, 2, 3)  # compute order: f, i, g, o

        # All gates use tanh (sigmoid(z) = (1+tanh(z/2))/2) so that every LUT
        # op on the Scalar engine -- tanh AND exp -- lives in the single
        # "exp_and_others" activation table: zero table reloads, and the
        # softmax tail interleaves freely with the per-step gate activations.
        # Scale bookkeeping: hist holds 4*h, sT holds 2*c, W_h is pre-divided
        # by 4 on the host, and the exp uses scale=0.25.

        def softmax_tail(tau):
            e = gpool.tile([H, BS], bf16, name=f"e_{tau}", tag="e", bufs=4)
            nc.scalar.activation(e, hist[tau % K][0:H, :], EXP, scale=0.25)
            # masked transpose: out[128b, 0:100] = (e chunk)^T * mask,
            # out[:, 100] = sum_h(e * mask)  -- mask and reduction on the PE
            eT = epool.tile([128, NB, 104], f32, name=f"eT_{tau}", tag="eT")
            for k in range(NB):
                nc.tensor.matmul(
                    eT[:, k, 0:MT],
                    e[:, 128 * k : 128 * (k + 1)],
                    mt[:, tau, :],
                    start=True,
                    stop=True,
                )
            r = opool.tile([128, NB], f32, name=f"r_{tau}", tag="r")
            nc.vector.reciprocal(r, eT[:, :, 100])
            ot = opool.tile([128, NB, H], f32, name=f"ot_{tau}", tag="ot")
            for k in range(NB):
                nc.vector.tensor_scalar_mul(
                    ot[:, k, :], eT[:, k, 0:H], r[:, k : k + 1]
                )
            nc.sync.dma_start(
                out=out_d[:, tau, :].rearrange("(k p) h -> p k h", p=128), in_=ot
            )

        for t in range(P):
            # ---- stream x_t^T in, feature chunks on partitions (one DMA) ----
            xtile = xpool.tile([128, XE // 128, BS], xdt, name=f"x_{t}", tag="x")
            nc.sync.dma_start(
                out=xtile, in_=xT_d[t].rearrange("(c p) b -> p c b", p=128)
            )

            # ---- z^T per gate (f, i, g, o), four PSUM banks ----
            zg = [None] * 4
            for wcol in GSLICE:
                z = zpool.tile([H, BS], f32, name=f"z_{t}_{wcol}", tag="z")
                if USE_FP8:
                    # DoubleRow: two 128-row k-tiles contracted per matmul at
                    # half cycle cost -> 2 instructions cover all 512 of E.
                    # (weight-cols, x-chunk-base) product terms: i,f,o use
                    # x8@W8; g adds r8@W8 and x8@S8 residual corrections
                    terms = [(wcol * H, 0)]
                    if wcol == 2:
                        terms += [(wcol * H, NE), (400, 0)]
                    first = True
                    for wof, xof in terms:
                        for cp in range(NE // 2):
                            nc.tensor.matmul(
                                z,
                                wxb[:, 2 * cp : 2 * cp + 2, wof : wof + H],
                                xtile[:, xof + 2 * cp : xof + 2 * cp + 2, :],
                                start=first,
                                stop=False,
                                perf_mode=DR,
                            )
                            first = False
                else:
                    for ec in range(NE):
                        nc.tensor.matmul(
                            z,
                            wxb[:, ec, wcol * H : (wcol + 1) * H],
                            xtile[:, ec, :],
                            start=(ec == 0),
                            stop=False,
                        )
                nc.tensor.matmul(
                    z,
                    whb[:, wcol * H : (wcol + 1) * H],
                    hist[(t - 1) % K],
                    start=False,
                    stop=True,
                )
                zg[wcol] = z

            # ---- gates: T = tanh(z/2); T_f first so u launches early ----
            Tf = gpool.tile([H, BS], bf16, name=f"Tf_{t}", tag="Tf", bufs=3)
            nc.scalar.activation(Tf, zg[1], TANH, scale=0.5)
            Ti = gpool.tile([H, BS], bf16, name=f"Ti_{t}", tag="Ti", bufs=3)
            nc.scalar.activation(Ti, zg[0], TANH, scale=0.5)
            To = gpool.tile([H, BS], bf16, name=f"To_{t}", tag="To", bufs=3)
            nc.scalar.activation(To, zg[3], TANH, scale=0.5)

            # fused elementwise chain (scalar_tensor_tensor):
            #   u = (1+Tf)*s      = 4 f c      (GpSimd, parallel with Ti->v)
            #   v = (1+Ti)*g      = 2 i g      (Vector; g is PSUM)
            #   s' = 0.5 u + v    = 2 c_new
            #   hh = (1+To)*s'    = 4 h_new
            u = gpool.tile([H, BS], bf16, name=f"u_{t}", tag="u", bufs=2)
            nc.vector.scalar_tensor_tensor(u, Tf, 1.0, sT, ADD, MULT)
            v = gpool.tile([H, BS], bf16, name=f"v_{t}", tag="v", bufs=2)
            nc.vector.scalar_tensor_tensor(v, Ti, 1.0, zg[2], ADD, MULT)
            nc.vector.scalar_tensor_tensor(sT, u, 0.5, v, MULT, ADD)
            nc.vector.scalar_tensor_tensor(
                hist[t % K][0:H, :], To, 1.0, sT, ADD, MULT
            )

            # lag the tail one step: exp(t-1) slots into the Scalar engine's
            # idle window right after To(t), never delaying the next tanh
            if t > 0:
                softmax_tail(t - 1)
        softmax_tail(P - 1)

    nc.compile()
    return nc


def _get_program():
    global _PROGRAM
    if _PROGRAM is None:
        _PROGRAM = _build_program()
    return _PROGRAM


def _prep_in_maps(h_enc, h0, W_x, W_h, b, mask):
    h_enc = np.asarray(h_enc, dtype=np.float32)
    h0 = np.asarray(h0, dtype=np.float32)
    W_x = np.asarray(W_x, dtype=np.float32)
    W_h = np.asarray(W_h, dtype=np.float32)
    b = np.asarray(b, dtype=np.float32)
    mask = np.asarray(mask)

    import ml_dtypes

    bf16 = ml_dtypes.bfloat16
    xdt = ml_dtypes.float8_e4m3 if USE_FP8 else bf16

    # lhsT layout for the xW matmuls: row p holds W_x[ec*128 + p, :] for the
    # 4 feature chunks side by side -> [128, 4*400]
    if USE_FP8:
        W8 = W_x.astype(xdt)
        S8g = (W_x[:, 200:300] - W8[:, 200:300].astype(np.float32)).astype(xdt)
        pad = np.zeros((E, 12), np.float32).astype(xdt)
        wpack = np.concatenate([W8, S8g, pad], axis=1)  # [512, 512]
        wxb = np.ascontiguousarray(
            wpack.reshape(NE, 128, 512).transpose(1, 0, 2).reshape(128, NE * 512)
        )
    else:
        wxb = np.ascontiguousarray(
            W_x.reshape(NE, 128, 400).transpose(1, 0, 2).reshape(128, NE * 400)
        ).astype(xdt)
    # hist holds 4*h -> W_h/4; bias row multiplies the constant-1.0 row
    whb = np.concatenate([W_h / 4.0, b[None, :]], axis=0).astype(bf16)
    # masked transpose matrices: [diag(mask_t) | mask_t] per step
    m01 = mask.astype(np.float32)  # [P, H]
    mt = np.zeros((H, P, MT), np.float32)
    for tau in range(P):
        mt[:, tau, 0:H] = np.diag(m01[tau])
        mt[:, tau, H] = m01[tau]
    mt = np.ascontiguousarray(mt.reshape(H, P * MT)).astype(bf16)

    in_maps = []
    xTf = np.empty((P, E, BS), np.float32)
    for c in range(NCORES):
        shard = h_enc[c * BS : (c + 1) * BS]  # [BS, P, E]
        for t in range(P):
            xTf[t] = shard[:, t, :].T
        if USE_FP8:
            x8 = xTf.astype(xdt)
            r8 = (xTf - x8.astype(np.float32)).astype(xdt)
            xT = np.ascontiguousarray(np.concatenate([x8, r8], axis=1))
        else:
            xT = xTf.astype(xdt)
        h0T = np.ascontiguousarray(
            np.concatenate(
                [4.0 * h0[c * BS : (c + 1) * BS].T, np.ones((1, BS), np.float32)],
                axis=0,
            )
        ).astype(bf16)
        in_maps.append({"xT": xT, "wxb": wxb, "whb": whb, "mt": mt, "h0T": h0T})
    return in_maps


def run(inputs: dict, trace: bool = False):
    """Run on 8 cores; returns (full_output, exec_time_ns_or_None)."""
    from concourse.bass_utils import run_bass_kernel_spmd

    nc = _get_program()
    in_maps = _prep_in_maps(**inputs)
    res = run_bass_kernel_spmd(
        nc, in_maps, core_ids=list(range(NCORES)), trace=trace
    )
    out = np.concatenate([r["out"] for r in res.results], axis=0)
    return out, res.exec_time_ns


def kernel(**inputs) -> np.ndarray:
    out, _ = run(inputs, trace=False)
    return out


# revision 36
# speedup vs baseline: 1.1886x; 1.1886x over previous
"""Trainium2 Bass kernel for the 81-step LSTM decoder + masked softmax.

Math (per batch row b):
    z_t = x_t @ W_x + h_{t-1} @ W_h + b          (gates i, f, g, o; 100 each)
    i,f,o = sigmoid;  g = identity
    c_t = f*c_{t-1} + i*g;  h_t = o*c_t
    out_t = softmax(where(mask_t, h_t, -inf))

Strategy: data-parallel over batch (4096 -> 8 cores x 512). Each core runs
an identical Bass program on its shard; no collectives.

Device layout is feature-major ("transposed"): the recurrent state h^T is
kept as [101, 512] bf16 (hidden-on-partitions, batch-on-free, +1 ones row so
the bias rides in an augmented weight row).  x is fed to the device already
transposed on the host as xT [81, 512e, 512b] so the contraction dim (e)
lands on partitions with zero on-device data transposes.  All z matmuls run
in bf16 (2 cols/cycle on the PE).  The softmax mask and the softmax row-sum
are folded into the per-step PE transpose: instead of an identity, the
transpose right-multiplies by [diag(mask_t) | mask_t], so the PE emits the
masked exp(h) batch-major AND its row sums (column 100) in one pass --
no mask bias on the exp and no vector-engine reduction.  Elementwise work
is split between the Vector and GpSimd engines.
"""

import sys

if "/opt/trn_rl_repo" not in sys.path:
    sys.path.insert(0, "/opt/trn_rl_repo")

import numpy as np

P = 81       # places / timesteps
H = 100      # LSTM units
E = 512      # encoder feature width
B = 4096     # total batch
NCORES = 8
BS = B // NCORES          # 512 batch rows per core
NB = BS // 128            # 4 batch tiles of 128
NE = E // 128             # 4 feature chunks of 128
K = 9                     # softmax/exp batching window (81 % 9 == 0)
MT = 101                  # masked-transpose matrix cols (100 + sum col)
# fp8-e4m3 DoubleRow for the x@W_x matmuls (2 k-tiles/instr, 2x col rate).
# Plain fp8 on every gate fails the 2e-2 gate (2.25e-2); with i,f,o in plain
# fp8 (sigmoid-damped) and the identity gate g residual-corrected
# (x8@W8 + r8@W8 + x8@S8), the simulated/measured rel err is ~1.47e-2.
USE_FP8 = True

_PROGRAM = None


def _build_program():
    import concourse.bacc as bacc
    import concourse.bass as bass
    import concourse.mybir as mybir
    from concourse.tile import TileContext
    from concourse.tile_rust import add_dep_helper
    from contextlib import ExitStack

    f32 = mybir.dt.float32
    bf16 = mybir.dt.bfloat16
    xdt = mybir.dt.float8e4 if USE_FP8 else bf16
    TANH = mybir.ActivationFunctionType.Tanh
    EXP = mybir.ActivationFunctionType.Exp
    ADD = mybir.AluOpType.add
    MULT = mybir.AluOpType.mult

    DR = mybir.MatmulPerfMode.DoubleRow if USE_FP8 else None
    XE = 2 * E if USE_FP8 else E          # x8 rows then r8 rows when fp8
    WCOL = 512 if USE_FP8 else 400        # W8 [0:400], S8_g [400:500], pad

    nc = bacc.Bacc(None, target_bir_lowering=False)

    xT_d = nc.dram_tensor("xT", [P, XE, BS], xdt, kind="ExternalInput")
    wxb_d = nc.dram_tensor("wxb", [128, NE * WCOL], xdt, kind="ExternalInput")
    whb_d = nc.dram_tensor("whb", [H + 1, 400], bf16, kind="ExternalInput")
    mt_d = nc.dram_tensor("mt", [H, P * MT], bf16, kind="ExternalInput")
    h0T_d = nc.dram_tensor("h0T", [H + 1, BS], bf16, kind="ExternalInput")
    out_d = nc.dram_tensor("out", [BS, P, H], f32, kind="ExternalOutput")

    with ExitStack() as ctx:
        tc = ctx.enter_context(TileContext(nc))
        consts = ctx.enter_context(tc.tile_pool(name="consts", bufs=1))
        xpool = ctx.enter_context(tc.tile_pool(name="xpool", bufs=12))
        gpool = ctx.enter_context(tc.tile_pool(name="gpool", bufs=2))
        opool = ctx.enter_context(tc.tile_pool(name="opool", bufs=8))
        zpool = ctx.enter_context(tc.tile_pool(name="zpool", bufs=6, space="PSUM"))
        epool = ctx.enter_context(tc.tile_pool(name="epool", bufs=2, space="PSUM"))

        wxb = consts.tile([128, NE, WCOL], xdt)
        nc.sync.dma_start(out=wxb, in_=wxb_d.rearrange("p (c w) -> p c w", w=WCOL))
        whb = consts.tile([H + 1, 400], bf16)
        nc.sync.dma_start(out=whb, in_=whb_d[:, :])
        mt = consts.tile([H, P, MT], bf16)
        nc.sync.dma_start(out=mt, in_=mt_d.rearrange("h (p m) -> h p m", m=MT))
        # ring of recurrent-state snapshots; row H holds the constant 1.0 that
        # multiplies the bias row of whb
        hist = [consts.tile([H + 1, BS], bf16, name=f"hist{j}") for j in range(K)]
        for j in range(K - 1):
            nc.sync.dma_start(out=hist[j][H : H + 1, :], in_=h0T_d[H : H + 1, :])
        nc.sync.dma_start(out=hist[K - 1], in_=h0T_d[:, :])
        # persistent cell state, stored scaled: sT = 2*c (bf16)
        sT = consts.tile([H, BS], bf16)
        nc.vector.memset(sT, 0.0)

        # W column order: i [0:100], f [100:200], g [200:300], o [300:400]
        GSLICE = (1, 0, 2, 3)  # compute order: f, i, g, o

        # All gates use tanh (sigmoid(z) = (1+tanh(z/2))/2) so that every LUT
        # op on the Scalar engine -- tanh AND exp -- lives in the single
        # "exp_and_others" activation table: zero table reloads, and the
        # softmax tail interleaves freely with the per-step gate activations.
        # Scale bookkeeping: hist holds 4*h, sT holds 2*c, W_h is pre-divided
        # by 4 on the host, and the exp uses scale=0.25.

        # softmax tail for step tau, split in two so each engine's in-order
        # queue only sees tail work in its natural idle window of step tau+1
        eTs = {}

        def tail_exp(tau):
            e = gpool.tile([H, BS], bf16, name=f"e_{tau}", tag="e", bufs=4)
            nc.scalar.activation(e, hist[tau % K][0:H, :], EXP, scale=0.25)
            return e

        def tail_transpose(tau, e):
            # masked transpose: out[128b, 0:100] = (e chunk)^T * mask,
            # out[:, 100] = sum_h(e * mask)  -- mask and reduction on the PE
            eT = epool.tile([128, NB, 104], f32, name=f"eT_{tau}", tag="eT")
            for k in range(NB):
                nc.tensor.matmul(
                    eT[:, k, 0:MT],
                    e[:, 128 * k : 128 * (k + 1)],
                    mt[:, tau, :],
                    start=True,
                    stop=True,
                )
            eTs[tau] = eT

        def tail_rest(tau):
            eT = eTs.pop(tau)
            r = opool.tile([128, NB], f32, name=f"r_{tau}", tag="r")
            nc.vector.reciprocal(r, eT[:, :, 100])
            ot = opool.tile([128, NB, H], f32, name=f"ot_{tau}", tag="ot")
            for k in range(NB):
                nc.vector.tensor_scalar_mul(
                    ot[:, k, :], eT[:, k, 0:H], r[:, k : k + 1]
                )
            nc.sync.dma_start(
                out=out_d[:, tau, :].rearrange("(k p) h -> p k h", p=128), in_=ot
            )

        for t in range(P):
            # ---- stream x_t^T in, feature chunks on partitions (one DMA) ----
            xtile = xpool.tile([128, XE // 128, BS], xdt, name=f"x_{t}", tag="x")
            nc.sync.dma_start(
                out=xtile, in_=xT_d[t].rearrange("(c p) b -> p c b", p=128)
            )

            # exp(t-1) first: ready at step start, fills the Scalar engine's
            # idle slot before this step's z banks complete
            e_prev = tail_exp(t - 1) if t > 0 else None

            # ---- z^T per gate (f, i, g, o), four PSUM banks ----
            zg = [None] * 4
            for wcol in GSLICE:
                z = zpool.tile([H, BS], f32, name=f"z_{t}_{wcol}", tag="z")
                if USE_FP8:
                    # DoubleRow: two 128-row k-tiles contracted per matmul at
                    # half cycle cost -> 2 instructions cover all 512 of E.
                    # (weight-cols, x-chunk-base) product terms: i,f,o use
                    # x8@W8; g adds r8@W8 and x8@S8 residual corrections
                    terms = [(wcol * H, 0)]
                    if wcol == 2:
                        terms += [(wcol * H, NE), (400, 0)]
                    first = True
                    for wof, xof in terms:
                        for cp in range(NE // 2):
                            nc.tensor.matmul(
                                z,
                                wxb[:, 2 * cp : 2 * cp + 2, wof : wof + H],
                                xtile[:, xof + 2 * cp : xof + 2 * cp + 2, :],
                                start=first,
                                stop=False,
                                perf_mode=DR,
                            )
                            first = False
                else:
                    for ec in range(NE):
                        nc.tensor.matmul(
                            z,
                            wxb[:, ec, wcol * H : (wcol + 1) * H],
                            xtile[:, ec, :],
                            start=(ec == 0),
                            stop=False,
                        )
                nc.tensor.matmul(
                    z,
                    whb[:, wcol * H : (wcol + 1) * H],
                    hist[(t - 1) % K],
                    start=False,
                    stop=True,
                )
                zg[wcol] = z

            # transposes of exp(t-1): PE runs them while waiting for h_t
            if e_prev is not None:
                tail_transpose(t - 1, e_prev)

            # ---- gates: T = tanh(z/2); T_f first so u launches early ----
            Tf = gpool.tile([H, BS], bf16, name=f"Tf_{t}", tag="Tf", bufs=3)
            nc.scalar.activation(Tf, zg[1], TANH, scale=0.5)
            Ti = gpool.tile([H, BS], bf16, name=f"Ti_{t}", tag="Ti", bufs=3)
            nc.scalar.activation(Ti, zg[0], TANH, scale=0.5)
            To = gpool.tile([H, BS], bf16, name=f"To_{t}", tag="To", bufs=3)
            nc.scalar.activation(To, zg[3], TANH, scale=0.5)

            # fused elementwise chain (scalar_tensor_tensor):
            #   u = (1+Tf)*s      = 4 f c      (GpSimd, parallel with Ti->v)
            #   v = (1+Ti)*g      = 2 i g      (Vector; g is PSUM)
            #   s' = 0.5 u + v    = 2 c_new
            #   hh = (1+To)*s'    = 4 h_new
            u = gpool.tile([H, BS], bf16, name=f"u_{t}", tag="u", bufs=2)
            nc.vector.scalar_tensor_tensor(u, Tf, 1.0, sT, ADD, MULT)
            v = gpool.tile([H, BS], bf16, name=f"v_{t}", tag="v", bufs=2)
            nc.vector.scalar_tensor_tensor(v, Ti, 1.0, zg[2], ADD, MULT)
            nc.vector.scalar_tensor_tensor(sT, u, 0.5, v, MULT, ADD)
            nc.vector.scalar_tensor_tensor(
                hist[t % K][0:H, :], To, 1.0, sT, ADD, MULT
            )

            # recip/scale/store of tau=t-1 land in the Vector engine's idle
            # window between this step's chain and the next step's u
            if t > 0:
                tail_rest(t - 1)
        e_last = tail_exp(P - 1)
        tail_transpose(P - 1, e_last)
        tail_rest(P - 1)

    nc.compile()
    return nc


def _get_program():
    global _PROGRAM
    if _PROGRAM is None:
        _PROGRAM = _build_program()
    return _PROGRAM


def _prep_in_maps(h_enc, h0, W_x, W_h, b, mask):
    h_enc = np.asarray(h_enc, dtype=np.float32)
    h0 = np.asarray(h0, dtype=np.float32)
    W_x = np.asarray(W_x, dtype=np.float32)
    W_h = np.asarray(W_h, dtype=np.float32)
    b = np.asarray(b, dtype=np.float32)
    mask = np.asarray(mask)

    import ml_dtypes

    bf16 = ml_dtypes.bfloat16
    xdt = ml_dtypes.float8_e4m3 if USE_FP8 else bf16

    # lhsT layout for the xW matmuls: row p holds W_x[ec*128 + p, :] for the
    # 4 feature chunks side by side -> [128, 4*400]
    if USE_FP8:
        W8 = W_x.astype(xdt)
        S8g = (W_x[:, 200:300] - W8[:, 200:300].astype(np.float32)).astype(xdt)
        pad = np.zeros((E, 12), np.float32).astype(xdt)
        wpack = np.concatenate([W8, S8g, pad], axis=1)  # [512, 512]
        wxb = np.ascontiguousarray(
            wpack.reshape(NE, 128, 512).transpose(1, 0, 2).reshape(128, NE * 512)
        )
    else:
        wxb = np.ascontiguousarray(
            W_x.reshape(NE, 128, 400).transpose(1, 0, 2).reshape(128, NE * 400)
        ).astype(xdt)
    # hist holds 4*h -> W_h/4; bias row multiplies the constant-1.0 row
    whb = np.concatenate([W_h / 4.0, b[None, :]], axis=0).astype(bf16)
    # masked transpose matrices: [diag(mask_t) | mask_t] per step
    m01 = mask.astype(np.float32)  # [P, H]
    mt = np.zeros((H, P, MT), np.float32)
    for tau in range(P):
        mt[:, tau, 0:H] = np.diag(m01[tau])
        mt[:, tau, H] = m01[tau]
    mt = np.ascontiguousarray(mt.reshape(H, P * MT)).astype(bf16)

    in_maps = []
    xTf = np.empty((P, E, BS), np.float32)
    for c in range(NCORES):
        shard = h_enc[c * BS : (c + 1) * BS]  # [BS, P, E]
        for t in range(P):
            xTf[t] = shard[:, t, :].T
        if USE_FP8:
            x8 = xTf.astype(xdt)
            r8 = (xTf - x8.astype(np.float32)).astype(xdt)
            xT = np.ascontiguousarray(np.concatenate([x8, r8], axis=1))
        else:
            xT = xTf.astype(xdt)
        h0T = np.ascontiguousarray(
            np.concatenate(
                [4.0 * h0[c * BS : (c + 1) * BS].T, np.ones((1, BS), np.float32)],
                axis=0,
            )
        ).astype(bf16)
        in_maps.append({"xT": xT, "wxb": wxb, "whb": whb, "mt": mt, "h0T": h0T})
    return in_maps


def run(inputs: dict, trace: bool = False):
    """Run on 8 cores; returns (full_output, exec_time_ns_or_None)."""
    from concourse.bass_utils import run_bass_kernel_spmd

    nc = _get_program()
    in_maps = _prep_in_maps(**inputs)
    res = run_bass_kernel_spmd(
        nc, in_maps, core_ids=list(range(NCORES)), trace=trace
    )
    out = np.concatenate([r["out"] for r in res.results], axis=0)
    return out, res.exec_time_ns


def kernel(**inputs) -> np.ndarray:
    out, _ = run(inputs, trace=False)
    return out


# revision 37
# speedup vs baseline: 1.1981x; 1.0080x over previous
"""Trainium2 Bass kernel for the 81-step LSTM decoder + masked softmax.

Math (per batch row b):
    z_t = x_t @ W_x + h_{t-1} @ W_h + b          (gates i, f, g, o; 100 each)
    i,f,o = sigmoid;  g = identity
    c_t = f*c_{t-1} + i*g;  h_t = o*c_t
    out_t = softmax(where(mask_t, h_t, -inf))

Strategy: data-parallel over batch (4096 -> 8 cores x 512). Each core runs
an identical Bass program on its shard; no collectives.

Device layout is feature-major ("transposed"): the recurrent state h^T is
kept as [101, 512] bf16 (hidden-on-partitions, batch-on-free, +1 ones row so
the bias rides in an augmented weight row).  x is fed to the device already
transposed on the host as xT [81, 512e, 512b] so the contraction dim (e)
lands on partitions with zero on-device data transposes.  All z matmuls run
in bf16 (2 cols/cycle on the PE).  The softmax mask and the softmax row-sum
are folded into the per-step PE transpose: instead of an identity, the
transpose right-multiplies by [diag(mask_t) | mask_t], so the PE emits the
masked exp(h) batch-major AND its row sums (column 100) in one pass --
no mask bias on the exp and no vector-engine reduction.  Elementwise work
is split between the Vector and GpSimd engines.
"""

import sys

if "/opt/trn_rl_repo" not in sys.path:
    sys.path.insert(0, "/opt/trn_rl_repo")

import numpy as np

P = 81       # places / timesteps
H = 100      # LSTM units
E = 512      # encoder feature width
B = 4096     # total batch
NCORES = 8
BS = B // NCORES          # 512 batch rows per core
NB = BS // 128            # 4 batch tiles of 128
NE = E // 128             # 4 feature chunks of 128
K = 9                     # softmax/exp batching window (81 % 9 == 0)
MT = 101                  # masked-transpose matrix cols (100 + sum col)
# fp8-e4m3 DoubleRow for the x@W_x matmuls (2 k-tiles/instr, 2x col rate).
# Plain fp8 on every gate fails the 2e-2 gate (2.25e-2); with i,f,o in plain
# fp8 (sigmoid-damped) and the identity gate g residual-corrected
# (x8@W8 + r8@W8 + x8@S8), the simulated/measured rel err is ~1.47e-2.
USE_FP8 = False

_PROGRAM = None


def _build_program():
    import concourse.bacc as bacc
    import concourse.bass as bass
    import concourse.mybir as mybir
    from concourse.tile import TileContext
    from concourse.tile_rust import add_dep_helper
    from contextlib import ExitStack

    f32 = mybir.dt.float32
    bf16 = mybir.dt.bfloat16
    xdt = mybir.dt.float8e4 if USE_FP8 else bf16
    TANH = mybir.ActivationFunctionType.Tanh
    EXP = mybir.ActivationFunctionType.Exp
    ADD = mybir.AluOpType.add
    MULT = mybir.AluOpType.mult

    DR = mybir.MatmulPerfMode.DoubleRow if USE_FP8 else None
    XE = 2 * E if USE_FP8 else E          # x8 rows then r8 rows when fp8
    WCOL = 512 if USE_FP8 else 400        # W8 [0:400], S8_g [400:500], pad

    nc = bacc.Bacc(None, target_bir_lowering=False)

    xT_d = nc.dram_tensor("xT", [P, XE, BS], xdt, kind="ExternalInput")
    wxb_d = nc.dram_tensor("wxb", [128, NE * WCOL], xdt, kind="ExternalInput")
    whb_d = nc.dram_tensor("whb", [H + 1, 400], bf16, kind="ExternalInput")
    mt_d = nc.dram_tensor("mt", [H, P * MT], bf16, kind="ExternalInput")
    h0T_d = nc.dram_tensor("h0T", [H + 1, BS], bf16, kind="ExternalInput")
    out_d = nc.dram_tensor("out", [BS, P, H], f32, kind="ExternalOutput")

    with ExitStack() as ctx:
        tc = ctx.enter_context(TileContext(nc))
        consts = ctx.enter_context(tc.tile_pool(name="consts", bufs=1))
        xpool = ctx.enter_context(tc.tile_pool(name="xpool", bufs=12))
        gpool = ctx.enter_context(tc.tile_pool(name="gpool", bufs=2))
        opool = ctx.enter_context(tc.tile_pool(name="opool", bufs=8))
        zpool = ctx.enter_context(tc.tile_pool(name="zpool", bufs=6, space="PSUM"))
        epool = ctx.enter_context(tc.tile_pool(name="epool", bufs=2, space="PSUM"))

        wxb = consts.tile([128, NE, WCOL], xdt)
        nc.sync.dma_start(out=wxb, in_=wxb_d.rearrange("p (c w) -> p c w", w=WCOL))
        whb = consts.tile([H + 1, 400], bf16)
        nc.sync.dma_start(out=whb, in_=whb_d[:, :])
        mt = consts.tile([H, P, MT], bf16)
        nc.sync.dma_start(out=mt, in_=mt_d.rearrange("h (p m) -> h p m", m=MT))
        # ring of recurrent-state snapshots; row H holds the constant 1.0 that
        # multiplies the bias row of whb
        hist = [consts.tile([H + 1, BS], bf16, name=f"hist{j}") for j in range(K)]
        for j in range(K - 1):
            nc.sync.dma_start(out=hist[j][H : H + 1, :], in_=h0T_d[H : H + 1, :])
        nc.sync.dma_start(out=hist[K - 1], in_=h0T_d[:, :])
        # persistent cell state, stored scaled: sT = 2*c (bf16)
        sT = consts.tile([H, BS], bf16)
        nc.vector.memset(sT, 0.0)

        # W column order: i [0:100], f [100:200], g [200:300], o [300:400]
        GSLICE = (1, 0, 2, 3)  # compute order: f, i, g, o

        # All gates use tanh (sigmoid(z) = (1+tanh(z/2))/2) so that every LUT
        # op on the Scalar engine -- tanh AND exp -- lives in the single
        # "exp_and_others" activation table: zero table reloads, and the
        # softmax tail interleaves freely with the per-step gate activations.
        # Scale bookkeeping: hist holds 4*h, sT holds 2*c, W_h is pre-divided
        # by 4 on the host, and the exp uses scale=0.25.

        # softmax tail for step tau, split in two so each engine's in-order
        # queue only sees tail work in its natural idle window of step tau+1
        eTs = {}

        def tail_exp(tau):
            e = gpool.tile([H, BS], bf16, name=f"e_{tau}", tag="e", bufs=4)
            nc.scalar.activation(e, hist[tau % K][0:H, :], EXP, scale=0.25)
            return e

        def tail_transpose(tau, e):
            # masked transpose: out[128b, 0:100] = (e chunk)^T * mask,
            # out[:, 100] = sum_h(e * mask)  -- mask and reduction on the PE
            eT = epool.tile([128, NB, 104], f32, name=f"eT_{tau}", tag="eT")
            for k in range(NB):
                nc.tensor.matmul(
                    eT[:, k, 0:MT],
                    e[:, 128 * k : 128 * (k + 1)],
                    mt[:, tau, :],
                    start=True,
                    stop=True,
                )
            eTs[tau] = eT

        def tail_rest(tau):
            eT = eTs.pop(tau)
            r = opool.tile([128, NB], f32, name=f"r_{tau}", tag="r")
            nc.vector.reciprocal(r, eT[:, :, 100])
            ot = opool.tile([128, NB, H], f32, name=f"ot_{tau}", tag="ot")
            for k in range(NB):
                nc.vector.tensor_scalar_mul(
                    ot[:, k, :], eT[:, k, 0:H], r[:, k : k + 1]
                )
            nc.sync.dma_start(
                out=out_d[:, tau, :].rearrange("(k p) h -> p k h", p=128), in_=ot
            )

        for t in range(P):
            # ---- stream x_t^T in, feature chunks on partitions (one DMA) ----
            xtile = xpool.tile([128, XE // 128, BS], xdt, name=f"x_{t}", tag="x")
            nc.sync.dma_start(
                out=xtile, in_=xT_d[t].rearrange("(c p) b -> p c b", p=128)
            )

            # exp(t-1) first: ready at step start, fills the Scalar engine's
            # idle slot before this step's z banks complete
            e_prev = tail_exp(t - 1) if t > 0 else None

            # ---- z^T per gate (f, i, g, o), four PSUM banks ----
            zg = [None] * 4
            for wcol in GSLICE:
                z = zpool.tile([H, BS], f32, name=f"z_{t}_{wcol}", tag="z")
                if USE_FP8:
                    # DoubleRow: two 128-row k-tiles contracted per matmul at
                    # half cycle cost -> 2 instructions cover all 512 of E.
                    # (weight-cols, x-chunk-base) product terms: i,f,o use
                    # x8@W8; g adds r8@W8 and x8@S8 residual corrections
                    terms = [(wcol * H, 0)]
                    if wcol == 2:
                        terms += [(wcol * H, NE), (400, 0)]
                    first = True
                    for wof, xof in terms:
                        for cp in range(NE // 2):
                            nc.tensor.matmul(
                                z,
                                wxb[:, 2 * cp : 2 * cp + 2, wof : wof + H],
                                xtile[:, xof + 2 * cp : xof + 2 * cp + 2, :],
                                start=first,
                                stop=False,
                                perf_mode=DR,
                            )
                            first = False
                else:
                    for ec in range(NE):
                        nc.tensor.matmul(
                            z,
                            wxb[:, ec, wcol * H : (wcol + 1) * H],
                            xtile[:, ec, :],
                            start=(ec == 0),
                            stop=False,
                        )
                nc.tensor.matmul(
                    z,
                    whb[:, wcol * H : (wcol + 1) * H],
                    hist[(t - 1) % K],
                    start=False,
                    stop=True,
                )
                zg[wcol] = z

            # transposes of exp(t-1): PE runs them while waiting for h_t
            if e_prev is not None:
                tail_transpose(t - 1, e_prev)

            # ---- gates: T = tanh(z/2); T_f first so u launches early ----
            Tf = gpool.tile([H, BS], bf16, name=f"Tf_{t}", tag="Tf", bufs=3)
            nc.scalar.activation(Tf, zg[1], TANH, scale=0.5)
            Ti = gpool.tile([H, BS], bf16, name=f"Ti_{t}", tag="Ti", bufs=3)
            nc.scalar.activation(Ti, zg[0], TANH, scale=0.5)
            To = gpool.tile([H, BS], bf16, name=f"To_{t}", tag="To", bufs=3)
            nc.scalar.activation(To, zg[3], TANH, scale=0.5)

            # fused elementwise chain (scalar_tensor_tensor):
            #   u = (1+Tf)*s      = 4 f c      (GpSimd, parallel with Ti->v)
            #   v = (1+Ti)*g      = 2 i g      (Vector; g is PSUM)
            #   s' = 0.5 u + v    = 2 c_new
            #   hh = (1+To)*s'    = 4 h_new
            u = gpool.tile([H, BS], bf16, name=f"u_{t}", tag="u", bufs=2)
            nc.vector.scalar_tensor_tensor(u, Tf, 1.0, sT, ADD, MULT)
            v = gpool.tile([H, BS], bf16, name=f"v_{t}", tag="v", bufs=2)
            nc.vector.scalar_tensor_tensor(v, Ti, 1.0, zg[2], ADD, MULT)
            nc.vector.scalar_tensor_tensor(sT, u, 0.5, v, MULT, ADD)
            nc.vector.scalar_tensor_tensor(
                hist[t % K][0:H, :], To, 1.0, sT, ADD, MULT
            )

            # recip/scale/store of tau=t-1 land in the Vector engine's idle
            # window between this step's chain and the next step's u
            if t > 0:
                tail_rest(t - 1)
        e_last = tail_exp(P - 1)
        tail_transpose(P - 1, e_last)
        tail_rest(P - 1)

    nc.compile()
    return nc


def _get_program():
    global _PROGRAM
    if _PROGRAM is None:
        _PROGRAM = _build_program()
    return _PROGRAM


def _prep_in_maps(h_enc, h0, W_x, W_h, b, mask):
    h_enc = np.asarray(h_enc, dtype=np.float32)
    h0 = np.asarray(h0, dtype=np.float32)
    W_x = np.asarray(W_x, dtype=np.float32)
    W_h = np.asarray(W_h, dtype=np.float32)
    b = np.asarray(b, dtype=np.float32)
    mask = np.asarray(mask)

    import ml_dtypes

    bf16 = ml_dtypes.bfloat16
    xdt = ml_dtypes.float8_e4m3 if USE_FP8 else bf16

    # lhsT layout for the xW matmuls: row p holds W_x[ec*128 + p, :] for the
    # 4 feature chunks side by side -> [128, 4*400]
    if USE_FP8:
        W8 = W_x.astype(xdt)
        S8g = (W_x[:, 200:300] - W8[:, 200:300].astype(np.float32)).astype(xdt)
        pad = np.zeros((E, 12), np.float32).astype(xdt)
        wpack = np.concatenate([W8, S8g, pad], axis=1)  # [512, 512]
        wxb = np.ascontiguousarray(
            wpack.reshape(NE, 128, 512).transpose(1, 0, 2).reshape(128, NE * 512)
        )
    else:
        wxb = np.ascontiguousarray(
            W_x.reshape(NE, 128, 400).transpose(1, 0, 2).reshape(128, NE * 400)
        ).astype(xdt)
    # hist holds 4*h -> W_h/4; bias row multiplies the constant-1.0 row
    whb = np.concatenate([W_h / 4.0, b[None, :]], axis=0).astype(bf16)
    # masked transpose matrices: [diag(mask_t) | mask_t] per step
    m01 = mask.astype(np.float32)  # [P, H]
    mt = np.zeros((H, P, MT), np.float32)
    for tau in range(P):
        mt[:, tau, 0:H] = np.diag(m01[tau])
        mt[:, tau, H] = m01[tau]
    mt = np.ascontiguousarray(mt.reshape(H, P * MT)).astype(bf16)

    in_maps = []
    xTf = np.empty((P, E, BS), np.float32)
    for c in range(NCORES):
        shard = h_enc[c * BS : (c + 1) * BS]  # [BS, P, E]
        for t in range(P):
            xTf[t] = shard[:, t, :].T
        if USE_FP8:
            x8 = xTf.astype(xdt)
            r8 = (xTf - x8.astype(np.float32)).astype(xdt)
            xT = np.ascontiguousarray(np.concatenate([x8, r8], axis=1))
        else:
            xT = xTf.astype(xdt)
        h0T = np.ascontiguousarray(
            np.concatenate(
                [4.0 * h0[c * BS : (c + 1) * BS].T, np.ones((1, BS), np.float32)],
                axis=0,
            )
        ).astype(bf16)
        in_maps.append({"xT": xT, "wxb": wxb, "whb": whb, "mt": mt, "h0T": h0T})
    return in_maps


def run(inputs: dict, trace: bool = False):
    """Run on 8 cores; returns (full_output, exec_time_ns_or_None)."""
    from concourse.bass_utils import run_bass_kernel_spmd

    nc = _get_program()
    in_maps = _prep_in_maps(**inputs)
    res = run_bass_kernel_spmd(
        nc, in_maps, core_ids=list(range(NCORES)), trace=trace
    )
    out = np.concatenate([r["out"] for r in res.results], axis=0)
    return out, res.exec_time_ns


def kernel(**inputs) -> np.ndarray:
    out, _ = run(inputs, trace=False)
    return out
